# revision 29
# baseline (speedup 1.0000x reference)
"""Trainium2 Bass kernel for nn_CapsuleLayer (grouped 5x5 capsule conv + 3-iter
dynamic routing with local softmax), data-parallel over batch N=8 across 8 cores.

Per-core "C layout": spatial positions on SBUF partitions, channels on free dims.
  h = hb_h*16 + p_h   (hb_h in [0,3), p_h in [0,16))
  w = hb_w*8  + p_w   (hb_w in [0,6), p_w in [0,8))
  partition p = p_h*8 + p_w  (128)
  free block hb = hb_h*6 + hb_w  (18)

u_hat: [p=128, (hb=18, ci=8, co=16, do=16)] bf16.  All routing contractions
(ci/co/do) are free-dim ops (tensor_tensor trees, free-broadcast via 0-stride
APs); the 5x5 spatial pools run on small [48, (ci, 52)] transposed tiles via
DMA.  Conv runs on PE as 2 stacked-tap bf16 matmuls (K=128: 4x4 tap block x
di, K=72: remaining 9 taps x di) per (ci, out-half), accumulated in PSUM.
"""

import numpy as np
import ml_dtypes
from contextlib import ExitStack

import concourse.bass as bass
import concourse.tile as tile
from concourse import bacc, mybir
from concourse.bass_utils import run_bass_kernel_spmd

F32 = mybir.dt.float32
BF16 = mybir.dt.bfloat16
AF = mybir.ActivationFunctionType
ALU = mybir.AluOpType
AX = mybir.AxisListType

CI, DI, CO, DO = 8, 8, 16, 16
H = W = 48
HP = WP = 52
HW = H * W
HB = 18
PAD = 2
ROUTING = 3
NEG = -3.0e38

# taps: K1 = (kh,kw) in [0,4)x[0,4): t = kh*4+kw, row t*8+di  -> 128 rows
#       K2 = kh=4,kw=0..5 (5 taps) then kh=0..4,kw=4 (4 taps) -> 72 rows
K1_TAPS = [(kh, kw) for kh in range(4) for kw in range(4)]
K2_TAPS = [(4, kw) for kw in range(5)] + [(kh, 4) for kh in range(4)]


GUARD = 128  # zero guard elements before/after the flat image (h zero-pad)


def _emit(nc):
    u_d = nc.dram_tensor("u", [CI, DI, H, W], BF16, kind="ExternalInput").ap()
    w1_d = nc.dram_tensor("w1", [128, CI, 256], BF16, kind="ExternalInput").ap()
    w2_d = nc.dram_tensor("w2", [72, CI, 256], BF16, kind="ExternalInput").ap()
    r0_d = nc.dram_tensor("r0c", [128, HB], F32, kind="ExternalInput").ap()
    v_d = nc.dram_tensor("v", [128, HB, CO, DO], F32, kind="ExternalOutput").ap()

    with tile.TileContext(nc) as tc, ExitStack() as ctx:
        const = ctx.enter_context(tc.tile_pool(name="const", bufs=1))
        convp = ctx.enter_context(tc.tile_pool(name="convp", bufs=2))
        patp1 = ctx.enter_context(tc.tile_pool(name="patp1", bufs=2))
        patp2 = ctx.enter_context(tc.tile_pool(name="patp2", bufs=2))
        psum = ctx.enter_context(tc.tile_pool(name="psum", bufs=6, space="PSUM"))
        big = ctx.enter_context(tc.tile_pool(name="big", bufs=1))
        ring = ctx.enter_context(tc.tile_pool(name="ring", bufs=1))
        bigf = ctx.enter_context(tc.tile_pool(name="bigf", bufs=1))
        sm = ctx.enter_context(tc.tile_pool(name="sm", bufs=1))
        poolt = ctx.enter_context(tc.tile_pool(name="poolt", bufs=1))
        dpool = ctx.enter_context(tc.tile_pool(name="dpool", bufs=2, space="DRAM"))

        # ---- persistent tiles ----
        uhat = big.tile([128, HB, CI, CO, DO], BF16, name="uhat")
        b_t = big.tile([128, HB, CI, CO], F32, name="b_t")
        w1_t = const.tile([128, CI, 256], BF16, name="w1_t")
        w2_t = const.tile([72, CI, 256], BF16, name="w2_t")
        r0_t = const.tile([128, HB], F32, name="r0_t")
        nc.sync.dma_start(w1_t[:], w1_d[:])
        nc.sync.dma_start(w2_t[:], w2_d[:])
        nc.sync.dma_start(r0_t[:], r0_d[:])

        # pool scratch (pads preset once; interiors overwritten each use)
        mph = poolt.tile([48, WP, CI], F32, name="mph")  # [h, (wpad, ci)]
        mpw = poolt.tile([48, HP, CI], F32, name="mpw")  # [w, (hpad, ci)]
        sph = poolt.tile([48, WP, CI], F32, name="sph")
        spw = poolt.tile([48, HP, CI], F32, name="spw")
        nc.vector.memset(mph[:], NEG)
        nc.vector.memset(mpw[:], NEG)
        nc.vector.memset(sph[:], 0.0)
        nc.vector.memset(spw[:], 0.0)
        eps_t = const.tile([128, 1], F32, name="eps_t")
        nc.vector.memset(eps_t[:], 1e-9)

        # =========== Stage 1: conv -> uhat ===========
        # flat image (pitch 48) with zero guards; tap (kh,kw) = one contiguous
        # 2304-read at offset (kh-2)*48+(kw-2); w-bleed columns zeroed with
        # plain per-tap memsets.  M-block = contiguous flat-hw 128-run, so
        # PSUM partitions are C-partitions (p = hw%128) and evac is contiguous.
        upads = []
        for par in range(2):
            t = convp.tile([DI, GUARD + HW + GUARD], BF16, name=f"upad{par}", bufs=1)
            nc.vector.memset(t[:], 0.0)
            upads.append(t)

        for ci in range(CI):
            uflat = upads[ci % 2]
            nc.sync.dma_start(
                uflat[:, GUARD : GUARD + HW], u_d[ci].rearrange("di h w -> di (h w)")
            )
            pat1 = patp1.tile([128, HW], BF16, name="pat1")
            pat2 = patp2.tile([72, HW], BF16, name="pat2")
            for t, (kh, kw) in enumerate(K1_TAPS):
                off = GUARD + (kh - PAD) * W + (kw - PAD)
                nc.sync.dma_start(
                    pat1[t * 8 : (t + 1) * 8, :], uflat[:, off : off + HW]
                )
            for t, (kh, kw) in enumerate(K2_TAPS):
                off = GUARD + (kh - PAD) * W + (kw - PAD)
                nc.sync.dma_start(
                    pat2[t * 8 : (t + 1) * 8, :], uflat[:, off : off + HW]
                )
            # per-tap w-bleed zeroing via tiny DMAs from the (zero) guard
            for pat, taps in ((pat1, K1_TAPS), (pat2, K2_TAPS)):
                for t, (kh, kw) in enumerate(taps):
                    dw = kw - PAD
                    if dw == 0:
                        continue
                    a = abs(dw)
                    pv = pat[t * 8 : (t + 1) * 8, :].rearrange(
                        "di (h w) -> di h w", h=H
                    )
                    zsrc = uflat[:, 0 : H * a].rearrange(
                        "di (h w) -> di h w", w=a
                    )
                    dst = pv[:, :, W - dw : W] if dw > 0 else pv[:, :, 0:a]
                    nc.sync.dma_start(dst, zsrc)

            for hb in range(HB):
                lhs1 = pat1[:, hb * 128 : (hb + 1) * 128]
                lhs2 = pat2[:, hb * 128 : (hb + 1) * 128]
                ps = psum.tile([128, CO, DO], F32, name="ps")
                nc.tensor.matmul(ps[:], lhs1, w1_t[:, ci, :], start=True, stop=False)
                nc.tensor.matmul(ps[:], lhs2, w2_t[:, ci, :], start=False, stop=True)
                if hb % 2 == 0:
                    nc.scalar.copy(uhat[:, hb, ci], ps[:])
                else:
                    nc.vector.tensor_copy(uhat[:, hb, ci], ps[:])

        # =========== routing helpers ===========
        def mw_int(pwt):
            return pwt[:, PAD : PAD + H, :]

        def pools(src_c, is_max, out_c):
            """src_c [128,(hb,ci)] f32 -> 5x5 'same' window max/sum -> out_c.
            Spatial reorders ride DRAM-side APs (flat hw raster); SBUF-side
            APs stay plain."""
            ph, pw = (mph, mpw) if is_max else (sph, spw)
            op = ALU.max if is_max else ALU.add
            md = dpool.tile([HW, CI], F32, name="pmd", tag="pmd")
            nc.sync.dma_start(
                md[:].rearrange("(hb p) ci -> p hb ci", hb=HB), src_c[:]
            )
            nc.sync.dma_start(
                ph[:, PAD : PAD + W, :],
                md[:].rearrange("(h w) ci -> h w ci", h=H),
            )
            l1 = sm.tile([48, 51, CI], F32, name="pl1", tag="p51")
            nc.vector.tensor_tensor(l1[:], ph[:, 0:51], ph[:, 1:52], op=op)
            l2 = sm.tile([48, 49, CI], F32, name="pl2", tag="p49")
            nc.vector.tensor_tensor(l2[:], l1[:, 0:49], l1[:, 2:51], op=op)
            l3 = sm.tile([48, 48, CI], F32, name="pl3", tag="p48")
            nc.vector.tensor_tensor(l3[:], l2[:, 0:48], ph[:, 4:52], op=op)
            mt = dpool.tile([48, 48, CI], F32, name="pmt", tag="pmt")
            nc.sync.dma_start(mt[:], l3[:])
            nc.sync.dma_start(
                pw[:, PAD : PAD + H, :], mt[:].rearrange("h w ci -> w h ci")
            )
            m1 = sm.tile([48, 51, CI], F32, name="pm1", tag="p51")
            nc.vector.tensor_tensor(m1[:], pw[:, 0:51], pw[:, 1:52], op=op)
            m2 = sm.tile([48, 49, CI], F32, name="pm2", tag="p49")
            nc.vector.tensor_tensor(m2[:], m1[:, 0:49], m1[:, 2:51], op=op)
            m3 = sm.tile([48, 48, CI], F32, name="pm3", tag="p48")
            nc.vector.tensor_tensor(m3[:], m2[:, 0:48], pw[:, 4:52], op=op)
            mo = dpool.tile([HW, CI], F32, name="pmo", tag="pmo")
            nc.sync.dma_start(
                mo[:].rearrange("(h w) ci -> w h ci", h=H), m3[:]
            )
            nc.sync.dma_start(
                out_c[:], mo[:].rearrange("(hb p) ci -> p hb ci", hb=HB)
            )

        def squash(p_c, out_bf, out_f32):
            """p_c [128,(hb,co,do)] f32 -> squash over do."""
            sq = bigf.tile([128, HB, CO, DO], BF16, name="sq", tag="half", bufs=2)
            nc.scalar.activation(sq[:], p_c[:], AF.Square)
            nsq = sm.tile([128, HB, CO], F32, name="nsq")
            nc.vector.tensor_reduce(nsq[:], sq[:], axis=AX.X, op=ALU.add)
            rs = sm.tile([128, HB, CO], F32, name="rs")
            nc.scalar.activation(rs[:], nsq[:], AF.Sqrt, bias=eps_t[:])
            d1 = sm.tile([128, HB, CO], F32, name="d1")
            nc.vector.tensor_scalar_add(d1[:], nsq[:], 1.0)
            d2 = sm.tile([128, HB, CO], F32, name="d2")
            nc.vector.tensor_tensor(d2[:], d1[:], rs[:], op=ALU.mult)
            rd = sm.tile([128, HB, CO], F32, name="rd")
            nc.vector.reciprocal(rd[:], d2[:])
            g2 = sm.tile([128, HB, CO], F32, name="g2")
            nc.vector.tensor_tensor(g2[:], nsq[:], rd[:], op=ALU.mult)
            g_b = g2[:].unsqueeze(3).broadcast_to([128, HB, CO, DO])
            if out_bf is not None:
                nc.vector.tensor_tensor(out_bf[:], p_c[:], g_b, op=ALU.mult)
            if out_f32 is not None:
                nc.vector.tensor_tensor(out_f32[:], p_c[:], g_b, op=ALU.mult)

        # =========== Stage 2: routing ===========
        for it in range(ROUTING):
            p_c = bigf.tile([128, HB, CO, DO], F32, name="p_c", tag="pbig", bufs=1)
            if it == 0:
                for hb in range(HB):
                    t1 = ring.tile([128, 4, CO, DO], BF16, name="ct1")
                    nc.vector.tensor_tensor(
                        t1[:], uhat[:, hb, 0:4], uhat[:, hb, 4:8], op=ALU.add
                    )
                    t2 = ring.tile([128, 2, CO, DO], BF16, name="ct2")
                    nc.vector.tensor_tensor(t2[:], t1[:, 0:2], t1[:, 2:4], op=ALU.add)
                    us0 = ring.tile([128, CO, DO], F32, name="us0")
                    nc.vector.tensor_tensor(us0[:], t2[:, 0], t2[:, 1], op=ALU.add)
                    r0b = r0_t[:, hb : hb + 1].broadcast_to([128, CO, DO])
                    nc.vector.tensor_tensor(p_c[:, hb], us0[:], r0b, op=ALU.mult)
            else:
                m0 = sm.tile([128, HB, CI], F32, name="m0")
                nc.vector.tensor_reduce(m0[:], b_t[:], axis=AX.X, op=ALU.max)
                bmax = sm.tile([128, HB, CI], F32, name="bmax")
                pools(m0, True, bmax)
                cs = bigf.tile([128, HB, CI, CO], F32, name="cs", tag="half", bufs=2)
                bm_b = bmax[:].unsqueeze(3).broadcast_to([128, HB, CI, CO])
                nc.vector.tensor_tensor(cs[:], b_t[:], bm_b, op=ALU.subtract)
                c_t = bigf.tile([128, HB, CI, CO], BF16, name="c_t", tag="qtr", bufs=2)
                nc.scalar.activation(c_t[:], cs[:], AF.Exp)
                s_t = sm.tile([128, HB, CI], F32, name="s_t")
                nc.vector.tensor_reduce(s_t[:], c_t[:], axis=AX.X, op=ALU.add)
                sumc = sm.tile([128, HB, CI], F32, name="sumc")
                pools(s_t, False, sumc)
                rcp = sm.tile([128, HB, CI], F32, name="rcp")
                nc.vector.reciprocal(rcp[:], sumc[:])
                r_t = bigf.tile([128, HB, CI, CO], BF16, name="r_t", tag="qtr", bufs=2)
                rcp_b = rcp[:].unsqueeze(3).broadcast_to([128, HB, CI, CO])
                nc.vector.tensor_tensor(r_t[:], c_t[:], rcp_b, op=ALU.mult)
                for hb in range(HB):
                    x = ring.tile([128, CI, CO, DO], BF16, name="x")
                    r_b = r_t[:, hb].unsqueeze(3).broadcast_to([128, CI, CO, DO])
                    nc.vector.tensor_tensor(x[:], uhat[:, hb], r_b, op=ALU.mult)
                    t1 = ring.tile([128, 4, CO, DO], BF16, name="ct1")
                    nc.vector.tensor_tensor(t1[:], x[:, 0:4], x[:, 4:8], op=ALU.add)
                    t2 = ring.tile([128, 2, CO, DO], BF16, name="ct2")
                    nc.vector.tensor_tensor(t2[:], t1[:, 0:2], t1[:, 2:4], op=ALU.add)
                    nc.vector.tensor_tensor(p_c[:, hb], t2[:, 0], t2[:, 1], op=ALU.add)

            if it < ROUTING - 1:
                v_bf = bigf.tile([128, HB, CO, DO], BF16, name="v_bf", tag="half", bufs=2)
                squash(p_c, v_bf, None)
                for hb in range(HB):
                    y = ring.tile([128, CI, CO, DO], BF16, name="y")
                    v_b = v_bf[:, hb].unsqueeze(1).broadcast_to([128, CI, CO, DO])
                    nc.vector.tensor_tensor(y[:], uhat[:, hb], v_b, op=ALU.mult)
                    e1 = ring.tile([128, CI, CO, 8], BF16, name="dt1")
                    nc.vector.tensor_tensor(
                        e1[:], y[:, :, :, 0:8], y[:, :, :, 8:16], op=ALU.add
                    )
                    e2 = ring.tile([128, CI, CO, 4], BF16, name="dt2")
                    nc.vector.tensor_tensor(
                        e2[:], e1[:, :, :, 0:4], e1[:, :, :, 4:8], op=ALU.add
                    )
                    e3 = ring.tile([128, CI, CO, 2], BF16, name="dt3")
                    nc.vector.tensor_tensor(
                        e3[:], e2[:, :, :, 0:2], e2[:, :, :, 2:4], op=ALU.add
                    )
                    if it == 0:
                        nc.vector.tensor_tensor(
                            b_t[:, hb], e3[:, :, :, 0], e3[:, :, :, 1], op=ALU.add
                        )
                    else:
                        db = ring.tile([128, CI, CO], BF16, name="db")
                        nc.vector.tensor_tensor(
                            db[:], e3[:, :, :, 0], e3[:, :, :, 1], op=ALU.add
                        )
                        nc.vector.tensor_tensor(
                            b_t[:, hb], b_t[:, hb], db[:], op=ALU.add
                        )
            else:
                v_f = p_c
                squash(p_c, None, v_f)
                nc.sync.dma_start(v_d[:], v_f[:])
    return nc


# ============================ host side ============================

_CACHE = {}


def _host_consts(w):
    w1 = np.zeros((128, CI, 256), ml_dtypes.bfloat16)
    w2 = np.zeros((72, CI, 256), ml_dtypes.bfloat16)
    # w: [Ci, Co*Do=256, Di, 5, 5] f32; lhsT row t*8+di, cols (ci, m)
    for t, (kh, kw) in enumerate(K1_TAPS):
        for di in range(DI):
            w1[t * 8 + di] = w[:, :, di, kh, kw].astype(ml_dtypes.bfloat16)
    for t, (kh, kw) in enumerate(K2_TAPS):
        for di in range(DI):
            w2[t * 8 + di] = w[:, :, di, kh, kw].astype(ml_dtypes.bfloat16)

    hw_cnt = np.zeros((H, W), np.float32)
    for h in range(H):
        for wv in range(W):
            ch = min(h + 2, H - 1) - max(h - 2, 0) + 1
            cw = min(wv + 2, W - 1) - max(wv - 2, 0) + 1
            hw_cnt[h, wv] = ch * cw
    r0 = 1.0 / (CO * hw_cnt)
    r0f = r0.reshape(HB, 128)  # hw = hb*128 + p
    r0c = np.ascontiguousarray(r0f.T)
    return w1, w2, r0c


def _get_nc():
    if "nc" not in _CACHE:
        nc = bacc.Bacc("TRN2", target_bir_lowering=False, debug=False, num_devices=8)
        _emit(nc)
        nc.compile()
        _CACHE["nc"] = nc
    return _CACHE["nc"]


def kernel(u, w):
    u = np.asarray(u, np.float32)
    N = u.shape[0]
    assert N == 8
    nc = _get_nc()
    w1, w2, r0c = _host_consts(np.asarray(w, np.float32))
    in_maps = [
        {"u": u[n].astype(ml_dtypes.bfloat16), "w1": w1, "w2": w2, "r0c": r0c}
        for n in range(N)
    ]
    res = run_bass_kernel_spmd(nc, in_maps, core_ids=list(range(N)))
    out = np.stack([res.results[n]["v"] for n in range(N)])  # [8, 128, HB, CO, DO]
    out = out.transpose(0, 3, 4, 2, 1)  # n co do hb p  (hw = hb*128 + p)
    return np.ascontiguousarray(out.reshape(N, CO, DO, H, W), dtype=np.float32)
